# revision 1
# baseline (speedup 1.0000x reference)
"""Soft decision-tree forward (nn_DTree) on 8 trn2 NeuronCores.

Strategy (pure data parallel, per the sharding hint):
  - shard x row-wise 8 ways; replicate the tiny tree params.
  - per core: z = [x|1|1] @ [W | -c_hi | -c_lo]^T via bf16 PE matmuls into
    fp32 PSUM, g = sigmoid(z) on ACT, then a level-by-level value-tree
    blend:
       V_k = g_k * (V_{k+1,L} - V_{k+1,R}) + V_{k+1,R}
    with nodes pre-permuted (level-major, left-children-first) so every
    level's children are two contiguous halves of the previous level.
  - engine split: sigmoid on ACT, the level-7 leaf blend (the largest
    elementwise stage) on Pool/GpSimd, levels 6..0 on DVE.  The three
    engines pipeline across tile groups.
  - x reaches the PE transposed via the DMA xbar: x is cast to bf16 into a
    [rows, 64] DRAM bounce (cols 0-31 = features, 32-33 = bias-ones slots),
    whose [rows/2, 128] view is xbar-transpose-legal (cols % 128 == 0).
    The transposed SBUF buffer holds two interleaved row-classes
    (rows 2j+s at partitions 64s+f); the resulting row permutation of the
    [128, tiles] output is undone on the host (128 KiB reorder).
"""

import numpy as np
import ml_dtypes

import concourse.bass as bass
import concourse.bacc as bacc
import concourse.tile as tile
from concourse import mybir
from concourse.bass_utils import run_bass_kernel_spmd

BF16 = ml_dtypes.bfloat16

F = 32
D = 8
NODES = 255
LEAVES = 256
N_FULL = 262144
N_CORES = 8
ROWS = N_FULL // N_CORES  # 32768 rows per core
SLOTS = 32
CHUNK = 4096

# level-major offsets of each level's gates inside the 255-column block
LEVEL_OFF = {7: 0, 6: 128, 5: 192, 4: 224, 3: 240, 2: 248, 1: 252, 0: 254}


def _orderings():
    """ord[k] = local node order at level k (left-children-first recursion)."""
    ordv = {0: [0]}
    for k in range(7):
        ordv[k + 1] = [2 * i for i in ordv[k]] + [2 * i + 1 for i in ordv[k]]
    col_nodes = []
    for k in range(7, -1, -1):
        base = 2 ** k - 1
        col_nodes += [base + i for i in ordv[k]]
    return ordv, np.array(col_nodes)


def host_prep(feature_importances, feature_splits, leaf_node_classes, slots):
    """Tiny-param preprocessing (O(8K) work): relu/sigmoid/c, node permutation,
    bf16 weight matrix with split bias rows, leaf-blend constants."""
    fi = np.asarray(feature_importances, np.float32).reshape(NODES, F)
    fs = np.asarray(feature_splits, np.float32).reshape(NODES, F)
    cls = np.asarray(leaf_node_classes, np.float32).reshape(LEAVES)

    W = np.maximum(fi, 0.0)
    S = 1.0 / (1.0 + np.exp(-fs))
    c = np.sum(W * S, axis=1)  # (NODES,)

    ordv, col_nodes = _orderings()
    Wp = W[col_nodes]          # (255, 32) permuted level-major
    cp = c[col_nodes]

    c_hi = cp.astype(BF16).astype(np.float32)
    c_lo = (cp - c_hi).astype(np.float32)

    wt = np.zeros((128, 256), BF16)
    for b in (0, 64):  # replicate for both row-class partition groups
        wt[b : b + F, 0:NODES] = Wp.T.astype(BF16)
        wt[b + F, 0:NODES] = (-c_hi).astype(BF16)
        wt[b + F + 1, 0:NODES] = (-c_lo).astype(BF16)

    o7 = np.array(ordv[7])
    delta = (cls[2 * o7] - cls[2 * o7 + 1]).astype(BF16)
    beta = cls[2 * o7 + 1].astype(BF16)
    # (node, slot) layout: value for node j replicated across `slots` columns
    db = np.zeros((128, 2 * slots * 128), BF16)
    db[:, : slots * 128] = np.repeat(delta, slots)[None, :]
    db[:, slots * 128 :] = np.repeat(beta, slots)[None, :]
    return wt, db




def build_nc(rows, slots, chunk, pw=8, gbufs=3, osplit=4, pslots=15, pcut=3):
    """Build the single-core Bass program (SPMD across the cores).

    pslots: slot-stripe width handled end-to-end by Pool (rest on DVE);
    pcut: lowest level of the striped phase - levels pcut-1..0 run on DVE
    over all slots.  Each engine owns its stripe through the whole tree, so
    Pool and DVE never wait on each other inside a group.
    """
    assert rows % 128 == 0 and chunk % 256 == 0 and rows % chunk == 0
    tiles = rows // 128
    assert tiles % slots == 0
    groups = tiles // slots
    bf = mybir.dt.bfloat16
    f32 = mybir.dt.float32
    A = mybir.AluOpType

    nc = bacc.Bacc()
    x_in = nc.dram_tensor("x", [F + 4, rows], f32, kind="ExternalInput")
    wt_in = nc.dram_tensor("wt", [128, 256], bf, kind="ExternalInput")
    db_in = nc.dram_tensor("db", [128, 2 * slots * 128], bf, kind="ExternalInput")
    out_dram = nc.dram_tensor("out", [128, tiles], f32, kind="ExternalOutput")

    n_chunks = rows // chunk
    tpc = chunk // 256  # tiles per class per chunk
    mcut = 2 ** pcut

    with tile.TileContext(nc) as tc:
        with (
            tc.tile_pool(name="consts", bufs=1) as consts,
            tc.tile_pool(name="xT", bufs=1) as xtp,
            tc.tile_pool(name="zps", bufs=2, space="PSUM") as zps,
            tc.tile_pool(name="gpool", bufs=gbufs) as gpool,
            tc.tile_pool(name="vD", bufs=1) as vDp,
            tc.tile_pool(name="vP", bufs=1) as vPp,
            tc.tile_pool(name="vJ", bufs=2) as vJp,
            tc.tile_pool(name="opool", bufs=1) as opool,
        ):
            # ---- constants ----
            wt_sb = consts.tile([128, 256], bf)
            nc.sync.dma_start(out=wt_sb[:], in_=wt_in[:])
            # db rides the ACT queue: a 16KB transfer would stall the SP
            # queue's x transposes right at the start
            dbt = consts.tile([128, 2 * slots * 128], bf)
            nc.scalar.dma_start(out=dbt[:], in_=db_in[:])
            # [128, node, slot] views of the leaf-blend constants
            dbc = dbt[:, 0 : slots * 128].rearrange("p (a b) -> p a b", b=slots)
            bbc = dbt[:, slots * 128 :].rearrange("p (a b) -> p a b", b=slots)

            # ---- x arrives host-transposed [36, rows] f32; one casting DMA
            # per chunk loads it feature-major bf16 straight into SBUF ----
            xTs = []
            for ci in range(n_chunks):
                sl = slice(ci * chunk, (ci + 1) * chunk)
                xt = xtp.tile([F + 4, chunk], bf, tag=f"xT{ci}")
                nc.gpsimd.dma_start(out=xt[:], in_=x_in[:, sl])
                xTs.append(xt)

            out_sb = opool.tile([128, tiles], f32)

            # global tile g -> (lhsT slice of xT chunk, matching-base rhs slice)
            tpc2 = chunk // 128  # row tiles per chunk
            def operands_of(g):
                ci, t = divmod(g, tpc2)
                c0 = t * 128
                return (
                    xTs[ci][0:34, c0 : c0 + 128],
                    wt_sb[0:34, 0:NODES],
                )

            stripes = [
                (nc.vector, slice(pslots, slots), slots - pslots, vDp),
                (nc.gpsimd, slice(0, pslots), pslots, vPp),
            ]

            for gi in range(groups):
                # g layout: [128, node, slot] - every tree slice is a flat
                # contiguous range, keeping DVE/Pool in the bf16 2x perf mode.
                g_t = gpool.tile([128, 256, slots], bf)
                for half in range(slots // pw):
                    zt = zps.tile([128, pw * 256], f32)
                    ztv = zt[:].rearrange("p (j c) -> p c j", c=256)
                    for j in range(pw):
                        g = gi * slots + half * pw + j
                        lhs, rhs = operands_of(g)
                        nc.tensor.matmul(
                            ztv[:, 0:NODES, j],
                            lhsT=lhs,
                            rhs=rhs,
                            start=True,
                            stop=True,
                        )
                    nc.scalar.activation(
                        out=g_t[:, 0:NODES, half * pw : (half + 1) * pw],
                        in_=ztv[:, 0:NODES, :],
                        func=mybir.ActivationFunctionType.Sigmoid,
                    )
                # ---- value tree, slot-striped across DVE and Pool ----
                # joint tile holding the level-`pcut` values for all slots
                vj = vJp.tile([128, mcut, slots], bf, tag="vj")
                for eng, ssl, sw, vpool in stripes:
                    if sw == 0:
                        continue
                    gs = g_t[:, :, ssl]          # [128, 256, sw]
                    # level 7: v = g7 * delta + beta (per-node consts)
                    v = vpool.tile([128, 128, sw], bf, tag="v7")
                    eng.tensor_tensor(v[:], gs[:, 0:128, :], dbc[:, :, ssl], A.mult)
                    eng.tensor_tensor(v[:], v[:], bbc[:, :, ssl], A.add)
                    for k in range(6, pcut - 1, -1):
                        m = 2 ** k
                        off = LEVEL_OFF[k]
                        vl = v[:, 0:m, :]
                        vr = v[:, m : 2 * m, :]
                        vn = (
                            vj[:, :, ssl]
                            if k == pcut
                            else vpool.tile([128, m, sw], bf, tag=f"v{k}")
                        )
                        dt = vpool.tile([128, m, sw], bf, tag=f"d{k}")
                        eng.tensor_tensor(dt[:], vl, vr, A.subtract)
                        eng.tensor_tensor(vn[:], gs[:, off : off + m, :], dt[:], A.mult)
                        eng.tensor_tensor(vn[:], vn[:], vr, A.add)
                        v = vn
                # levels pcut-1..0 for all slots on DVE
                v = vj
                for k in range(pcut - 1, -1, -1):
                    m = 2 ** k
                    off = LEVEL_OFF[k]
                    vl = v[:, 0:m, :]
                    vr = v[:, m : 2 * m, :]
                    dt = vJp.tile([128, m, slots], bf, tag=f"dj{k}")
                    nc.vector.tensor_tensor(dt[:], vl, vr, A.subtract)
                    gk = g_t[:, off : off + m, :]
                    if k > 0:
                        vn = vJp.tile([128, m, slots], bf, tag=f"vj{k}")
                        nc.vector.tensor_tensor(vn[:], gk, dt[:], A.mult)
                        nc.vector.tensor_tensor(vn[:], vn[:], vr, A.add)
                        v = vn
                    else:
                        vo = out_sb[:, gi * slots : (gi + 1) * slots]
                        g0v = gk.rearrange("p a b -> p (a b)")
                        d0v = dt[:].rearrange("p a b -> p (a b)")
                        r0v = vr.rearrange("p a b -> p (a b)")
                        nc.vector.tensor_tensor(vo, g0v, d0v, A.mult)
                        nc.vector.tensor_tensor(vo, vo, r0v, A.add)

            step = max(1, groups // osplit)
            for g0 in range(0, groups, step):
                c0, c1 = g0 * slots, min((g0 + step) * slots, tiles)
                nc.sync.dma_start(out=out_dram[:, c0:c1], in_=out_sb[:, c0:c1])
    return nc


_CACHE = {}


def _get_nc(rows, slots=SLOTS, chunk=CHUNK):
    key = (rows, slots, chunk)
    if key not in _CACHE:
        nc = build_nc(rows, slots, chunk)
        if not nc.is_finalized():
            nc.finalize()
        _CACHE[key] = nc
    return _CACHE[key]


def run_device(xT, wt, db, slots=SLOTS, chunk=CHUNK, n_cores=N_CORES, trace=False):
    rows = xT.shape[1] // n_cores
    nc = _get_nc(rows, slots, chunk)
    in_maps = [
        {
            "x": np.ascontiguousarray(xT[:, i * rows : (i + 1) * rows]),
            "wt": wt,
            "db": db,
        }
        for i in range(n_cores)
    ]
    res = run_bass_kernel_spmd(nc, in_maps, list(range(n_cores)), trace=trace)
    out = np.empty((n_cores * rows, 1), np.float32)
    for i in range(n_cores):
        dev = res.results[i]["out"].astype(np.float32)  # [128, tiles]
        out[i * rows : (i + 1) * rows, 0] = dev.T.reshape(-1)
    return out, res


def pad_x(x):
    """Host staging: pad to 36 features (2 bias-ones + 2 zero) and
    transpose to feature-major [36, N]."""
    xp = np.zeros((F + 4, x.shape[0]), np.float32)
    xp[0:F, :] = x.T
    xp[F : F + 2, :] = 1.0
    return xp


def kernel(**inputs):
    x = pad_x(np.asarray(inputs["x"], np.float32).reshape(-1, F))
    wt, db = host_prep(
        inputs["feature_importances"],
        inputs["feature_splits"],
        inputs["leaf_node_classes"],
        SLOTS,
    )
    out, _ = run_device(x, wt, db, SLOTS, CHUNK)
    return out



# revision 24
# speedup vs baseline: 1.2460x; 1.2460x over previous
"""Soft decision-tree forward (nn_DTree) on 8 trn2 NeuronCores.

Strategy (pure data parallel): shard x row-wise 8 ways, replicate tree params.

Per core (32768 rows = 256 blocks of 128 rows):
  - One 255-col GEMM per block: z = [x|1|1] @ [W | -c_hi | -c_lo]^T into a
    persistent PSUM tile (two 4-bank halves alternate per 8-block group;
    range-granular WAR tracking keeps the next-next group's GEMMs off the
    sigmoid's critical path).
  - ONE sigmoid per group (fe=2040) -> bf16 g in SBUF.  g is laid out at a
    uniform 255-col stride per block across a 32-block super-tile, so every
    blend op batches 32 blocks with a 3D access pattern.
  - Value-tree blend (level-major, left-children-first permutation):
    levels 7..3 striped across DVE (2x bf16) and Pool by block ranges,
    levels 2..0 (tiny) entirely on Pool.  Level 7 blends with broadcast
    delta/beta const tiles; output written bf16.
  - x arrives host-transposed bf16 [34, rows]; output [128, 256] bf16,
    unpacked on host.
"""

import numpy as np
import ml_dtypes

import concourse.bass as bass
import concourse.bacc as bacc
import concourse.tile as tile
from concourse import mybir
from concourse.bass_utils import run_bass_kernel_spmd

BF16 = ml_dtypes.bfloat16

F = 32
D = 8
NODES = 255
LEAVES = 256
N_FULL = 262144
N_CORES = 8
ROWS = N_FULL // N_CORES  # 32768 rows per core
SLOTS = 32                # kept for test.py compat
CHUNK = 4096              # kept for test.py compat

K = 8                                  # blocks per σ-group
SUPERS = [1, 1, 2] + [4] * 6 + [2, 1, 1]  # front/back taper
DVE_CUT = 155 / 256                    # fraction of blocks on DVE for L7..3

# level-major offsets of each level's gates inside the 255-col block
LEVEL_OFF = {7: 0, 6: 128, 5: 192, 4: 224, 3: 240, 2: 248, 1: 252, 0: 254}


def _orderings():
    """ord[k] = local node order at level k (left-children-first recursion)."""
    ordv = {0: [0]}
    for k in range(7):
        ordv[k + 1] = [2 * i for i in ordv[k]] + [2 * i + 1 for i in ordv[k]]
    col_nodes = []
    for k in range(7, -1, -1):
        base = 2 ** k - 1
        col_nodes += [base + i for i in ordv[k]]
    return ordv, np.array(col_nodes)


def host_prep(feature_importances, feature_splits, leaf_node_classes, slots=SLOTS):
    """relu/sigmoid/c, node permutation, bf16 weights with split bias rows,
    leaf-blend delta/beta broadcast constants."""
    fi = np.asarray(feature_importances, np.float32).reshape(NODES, F)
    fs = np.asarray(feature_splits, np.float32).reshape(NODES, F)
    cls = np.asarray(leaf_node_classes, np.float32).reshape(LEAVES)

    W = np.maximum(fi, 0.0)
    S = 1.0 / (1.0 + np.exp(-fs))
    c = np.sum(W * S, axis=1)  # (NODES,)

    ordv, col_nodes = _orderings()
    Wp = W[col_nodes]          # (255, 32) permuted level-major
    cp = c[col_nodes]

    c_hi = cp.astype(BF16).astype(np.float32)
    c_lo = (cp - c_hi).astype(np.float32)

    wt = np.zeros((F + 2, 256), BF16)
    wt[0:F, 0:NODES] = Wp.T.astype(BF16)
    wt[F, 0:NODES] = (-c_hi).astype(BF16)
    wt[F + 1, 0:NODES] = (-c_lo).astype(BF16)

    o7 = np.array(ordv[7])
    delta = (cls[2 * o7] - cls[2 * o7 + 1]).astype(BF16)
    beta = cls[2 * o7 + 1].astype(BF16)
    # [128, block, node] broadcast tiles (replicated across 40 blocks = max
    # stripe width; blend slices only use the replica COUNT, not position)
    db = np.zeros((128, 2 * 40 * 128), BF16)
    db[:, 0:40 * 128] = np.tile(delta, 40)[None, :]
    db[:, 40 * 128:] = np.tile(beta, 40)[None, :]
    return wt, db


def build_nc(rows, k_blk=K, supers=None, dve_cut=DVE_CUT):
    if supers is None:
        supers = SUPERS
    assert rows % (128 * k_blk) == 0
    blocks = rows // 128
    groups = blocks // k_blk
    assert sum(supers) == groups
    bf = mybir.dt.bfloat16
    f32 = mybir.dt.float32
    A = mybir.AluOpType

    nc = bacc.Bacc()
    x_in = nc.dram_tensor("x", [F + 2, rows], bf, kind="ExternalInput")
    wt_in = nc.dram_tensor("wt", [F + 2, 256], bf, kind="ExternalInput")
    db_in = nc.dram_tensor("db", [128, 2 * 40 * 128], bf, kind="ExternalInput")
    out_dram = nc.dram_tensor("out", [128, blocks], bf, kind="ExternalOutput")

    GW = 255 * k_blk  # 2040 cols per group slab

    with tile.TileContext(nc) as tc:
        with (
            tc.tile_pool(name="consts", bufs=1) as consts,
            tc.tile_pool(name="xT", bufs=1) as xtp,
            tc.tile_pool(name="zps", bufs=1, space="PSUM") as zps,
            tc.tile_pool(name="gpool", bufs=3) as gpool,
            tc.tile_pool(name="blend", bufs=2) as blp,
            tc.tile_pool(name="v3pool", bufs=2) as v3p,
            tc.tile_pool(name="opool", bufs=1) as opool,
        ):
            # ---- constants ----
            wt_sb = consts.tile([F + 2, 256], bf)
            nc.gpsimd.dma_start(out=wt_sb[:], in_=wt_in[:])
            dbt = consts.tile([128, 2 * 40 * 128], bf)
            # warmup: preload the sigmoid ACT table before real data arrives
            warm = consts.tile([128, 1], f32)
            nc.vector.memset(warm[:], 0.0)
            wsig = consts.tile([128, 1], bf)
            nc.scalar.activation(out=wsig[:], in_=warm[:],
                                 func=mybir.ActivationFunctionType.Sigmoid)
            dbc = dbt[:, 0:40 * 128].rearrange("p (b n) -> p b n", n=128)
            bbc = dbt[:, 40 * 128:].rearrange("p (b n) -> p b n", n=128)

            # ---- x: feature-major bf16, pooled chunks on the SP queue; the
            # big db broadcast tile loads after the first x chunks ----
            xchunks = []   # (col0, cols, tile)
            off = 0
            n_chunk = 0
            while off < rows:
                cs = 2048 if n_chunk < 2 else 4096
                cs = min(cs, rows - off)
                cxt = xtp.tile([F + 2, cs], bf, tag=f"x{cs}", name=f"xc{n_chunk}",
                               bufs=(2 if cs == 2048 else 4))
                nc.sync.dma_start(out=cxt[:], in_=x_in[:, off:off + cs])
                xchunks.append((off, cs, cxt))
                off += cs
                n_chunk += 1
                if n_chunk == 2:
                    nc.gpsimd.dma_start(out=dbt[:, 0:5120], in_=db_in[:, 0:5120])
                    nc.gpsimd.dma_start(out=dbt[:, 5120:], in_=db_in[:, 5120:])

            def x_slice(b):
                c0 = b * 128
                for off_, cs_, t_ in xchunks:
                    if off_ <= c0 < off_ + cs_:
                        return t_[:, c0 - off_:c0 - off_ + 128]
                raise AssertionError

            out_sb = opool.tile([128, blocks], bf)
            # PE p-state warmup: dummy matmuls keep PE busy during x load
            dmy = consts.tile([128, 512], bf)
            nc.vector.memset(dmy[:], 0.0)
            # persistent PSUM tile; two 2048-col halves alternate per group
            zbig = zps.tile([128, 4096], f32)
            for w in range(5):
                nc.tensor.matmul(zbig[:, 2048:2560], lhsT=dmy[:, 0:128],
                                 rhs=dmy[:], start=True, stop=True)

            def emit_z(g):
                zt = zbig[:, (g % 2) * 2048:(g % 2) * 2048 + GW]
                b0 = g * k_blk
                for j in range(k_blk):
                    xs = x_slice(b0 + j)
                    nc.tensor.matmul(
                        zt[:, 255 * j:255 * (j + 1)],
                        lhsT=xs, rhs=wt_sb[:, 0:255],
                        start=True, stop=True)

            sup_of_group = []
            for si, sg in enumerate(supers):
                sup_of_group += [si] * sg
            g_tiles = {}

            def emit_sig(g):
                si = sup_of_group[g]
                sg = supers[si]
                if si not in g_tiles:
                    g_tiles[si] = gpool.tile(
                        [128, GW * sg], bf, tag=f"g{sg}", name=f"gsup{si}",
                        bufs=(3 if sg == 4 else (1 if sg == 2 else 2)))
                q = g - sup_of_group.index(si)
                zt = zbig[:, (g % 2) * 2048:(g % 2) * 2048 + GW]
                nc.scalar.activation(
                    out=g_tiles[si][:, q * GW:(q + 1) * GW], in_=zt,
                    func=mybir.ActivationFunctionType.Sigmoid)

            def emit_blend(si, g0):
                """blend for super si covering blocks [g0*K, g0*K + sb*K)."""
                sg = supers[si]
                sb = sg * k_blk                      # blocks in this super
                b0 = g0 * k_blk
                gt = g_tiles[si]
                gv = gt[:, 0:sb * 255].rearrange("p (b c) -> p b c", c=255)
                cut = max(1, min(sb - 1, round(dve_cut * sb)))
                stripes = [
                    (nc.vector, 0, cut, "dv"),
                    (nc.gpsimd, cut, sb, "pl"),
                ]
                v3s = v3p.tile([128, sb, 8], bf, tag=f"v3_{sg}", name="v3s", bufs=(1 if sg == 2 else 2))
                for eng, s0, s1, nm in stripes:
                    sw = s1 - s0
                    gs = gv[:, s0:s1, :]
                    # L7: v = g7*delta + beta
                    vt = blp.tile([128, sw, 128], bf, tag=f"v7{nm}_{sg}", name="vt", bufs=(1 if sg == 2 else 2))
                    v = vt[:, 0:sw, :]
                    eng.tensor_tensor(v, gs[:, :, 0:128], dbc[:, 0:sw, :], A.mult)
                    eng.tensor_tensor(v, v, bbc[:, 0:sw, :], A.add)
                    for k in range(6, 2, -1):
                        m = 2 ** k
                        off_ = LEVEL_OFF[k]
                        vl = v[:, :, 0:m]
                        vr = v[:, :, m:2 * m]
                        if k == 3:
                            vn = v3s[:, s0:s1, :]
                        else:
                            vnt = blp.tile([128, sw, m], bf, tag=f"v{k}{nm}_{sg}", name="vnt", bufs=(1 if sg == 2 else 2))
                            vn = vnt[:, 0:sw, :]
                        dtt = blp.tile([128, sw, m], bf, tag=f"d{k}{nm}_{sg}", name="dtt", bufs=(1 if sg == 2 else 2))
                        dt_ = dtt[:, 0:sw, :]
                        eng.tensor_tensor(dt_, vl, vr, A.subtract)
                        eng.tensor_tensor(vn, gs[:, :, off_:off_ + m], dt_, A.mult)
                        eng.tensor_tensor(vn, vn, vr, A.add)
                        v = vn if k > 3 else None
                # levels 2..0 all on Pool
                v = v3s[:, 0:sb, :]
                for k in range(2, -1, -1):
                    m = 2 ** k
                    off_ = LEVEL_OFF[k]
                    vl = v[:, :, 0:m]
                    vr = v[:, :, m:2 * m]
                    dtt = blp.tile([128, sb, m], bf, tag=f"dj{k}_{sg}", name="dtt", bufs=(1 if sg == 2 else 2))
                    dt_ = dtt[:, 0:sb, :]
                    nc.gpsimd.tensor_tensor(dt_, vl, vr, A.subtract)
                    gk = gv[:, :, LEVEL_OFF[k]:LEVEL_OFF[k] + m]
                    if k > 0:
                        vnt = blp.tile([128, sb, m], bf, tag=f"vj{k}_{sg}", name="vnt", bufs=(1 if sg == 2 else 2))
                        vn = vnt[:, 0:sb, :]
                        nc.gpsimd.tensor_tensor(vn, gk, dt_, A.mult)
                        nc.gpsimd.tensor_tensor(vn, vn, vr, A.add)
                        v = vn
                    else:
                        vo = out_sb[:, b0:b0 + sb]
                        g0v = gk.rearrange("p a b -> p (a b)")
                        d0v = dt_.rearrange("p a b -> p (a b)")
                        r0v = vr.rearrange("p a b -> p (a b)")
                        nc.gpsimd.tensor_tensor(vo, g0v, d0v, A.mult)
                        nc.gpsimd.tensor_tensor(vo, vo, r0v, A.add)
                nc.sync.dma_start(out=out_dram[:, b0:b0 + sb], in_=out_sb[:, b0:b0 + sb])

            # software-pipelined emission: z one group ahead of σ
            emit_z(0)
            sup_start = {}
            for g in range(groups):
                si = sup_of_group[g]
                if si not in sup_start:
                    sup_start[si] = g
                if g + 1 < groups:
                    emit_z(g + 1)
                emit_sig(g)
                if g == sup_start[si] + supers[si] - 1:
                    emit_blend(si, sup_start[si])

    return nc


_CACHE = {}


def _get_nc(rows, slots=SLOTS, chunk=CHUNK):
    key = (rows,)
    if key not in _CACHE:
        nc = build_nc(rows)
        if not nc.is_finalized():
            nc.finalize()
        _CACHE[key] = nc
    return _CACHE[key]


def run_device(xT, wt, db, slots=SLOTS, chunk=CHUNK, n_cores=N_CORES, trace=False):
    rows = xT.shape[1] // n_cores
    nc = _get_nc(rows)
    in_maps = [
        {
            "x": np.ascontiguousarray(xT[:, i * rows:(i + 1) * rows]),
            "wt": wt,
            "db": db,
        }
        for i in range(n_cores)
    ]
    res = run_bass_kernel_spmd(nc, in_maps, list(range(n_cores)), trace=trace)
    out = np.empty((n_cores * rows, 1), np.float32)
    for i in range(n_cores):
        dev = res.results[i]["out"].astype(np.float32)  # [128, blocks]
        out[i * rows:(i + 1) * rows, 0] = dev.T.reshape(-1)
    return out, res


def pad_x(x):
    """Host staging: transpose to feature-major, two ones rows, cast bf16."""
    n = x.shape[0]
    xp = np.ones((F + 2, n), np.float32)
    xp[0:F, :] = x.T
    return xp.astype(BF16)


def kernel(**inputs):
    x = np.asarray(inputs["x"], np.float32).reshape(-1, F)
    wt, db = host_prep(
        inputs["feature_importances"],
        inputs["feature_splits"],
        inputs["leaf_node_classes"],
    )
    xT = pad_x(x)
    out, _ = run_device(xT, wt, db)
    return out


# revision 29
# speedup vs baseline: 1.2545x; 1.0068x over previous
"""Soft decision-tree forward (nn_DTree) on 8 trn2 NeuronCores.

Strategy (pure data parallel): shard x row-wise 8 ways, replicate tree params.

Per core (32768 rows = 256 blocks of 128 rows):
  - One 255-col GEMM per block: z = [x|1|1] @ [W | -c_hi | -c_lo]^T into a
    persistent PSUM tile (two 4-bank halves alternate per 8-block group;
    range-granular WAR tracking keeps the next-next group's GEMMs off the
    sigmoid's critical path).
  - ONE sigmoid per group (fe=2040) -> bf16 g in SBUF.  g is laid out at a
    uniform 255-col stride per block across a 32-block super-tile, so every
    blend op batches 32 blocks with a 3D access pattern.
  - Value-tree blend (level-major, left-children-first permutation):
    levels 7..3 striped across DVE (2x bf16) and Pool by block ranges,
    levels 2..0 (tiny) entirely on Pool.  Level 7 blends with broadcast
    delta/beta const tiles; output written bf16.
  - x arrives host-transposed bf16 [34, rows]; output [128, 256] bf16,
    unpacked on host.
"""

import numpy as np
import ml_dtypes

import concourse.bass as bass
import concourse.bacc as bacc
import concourse.tile as tile
from concourse import mybir
from concourse.bass_utils import run_bass_kernel_spmd

BF16 = ml_dtypes.bfloat16

F = 32
D = 8
NODES = 255
LEAVES = 256
N_FULL = 262144
N_CORES = 8
ROWS = N_FULL // N_CORES  # 32768 rows per core
SLOTS = 32                # kept for test.py compat
CHUNK = 4096              # kept for test.py compat

K = 8                                  # blocks per σ-group
SUPERS = [1, 1, 1, 1, 2, 2, 4, 4, 4, 4, 4, 2, 1, 1]  # fine taper
DVE_CUT = 155 / 256                    # fraction of blocks on DVE for L7..3

# level-major offsets of each level's gates inside the 255-col block
LEVEL_OFF = {7: 0, 6: 128, 5: 192, 4: 224, 3: 240, 2: 248, 1: 252, 0: 254}


def _orderings():
    """ord[k] = local node order at level k (left-children-first recursion)."""
    ordv = {0: [0]}
    for k in range(7):
        ordv[k + 1] = [2 * i for i in ordv[k]] + [2 * i + 1 for i in ordv[k]]
    col_nodes = []
    for k in range(7, -1, -1):
        base = 2 ** k - 1
        col_nodes += [base + i for i in ordv[k]]
    return ordv, np.array(col_nodes)


def host_prep(feature_importances, feature_splits, leaf_node_classes, slots=SLOTS):
    """relu/sigmoid/c, node permutation, bf16 weights with split bias rows,
    leaf-blend delta/beta broadcast constants."""
    fi = np.asarray(feature_importances, np.float32).reshape(NODES, F)
    fs = np.asarray(feature_splits, np.float32).reshape(NODES, F)
    cls = np.asarray(leaf_node_classes, np.float32).reshape(LEAVES)

    W = np.maximum(fi, 0.0)
    S = 1.0 / (1.0 + np.exp(-fs))
    c = np.sum(W * S, axis=1)  # (NODES,)

    ordv, col_nodes = _orderings()
    Wp = W[col_nodes]          # (255, 32) permuted level-major
    cp = c[col_nodes]

    c_hi = cp.astype(BF16).astype(np.float32)
    c_lo = (cp - c_hi).astype(np.float32)

    wt = np.zeros((F + 2, 256), BF16)
    wt[0:F, 0:NODES] = Wp.T.astype(BF16)
    wt[F, 0:NODES] = (-c_hi).astype(BF16)
    wt[F + 1, 0:NODES] = (-c_lo).astype(BF16)

    o7 = np.array(ordv[7])
    delta = (cls[2 * o7] - cls[2 * o7 + 1]).astype(BF16)
    beta = cls[2 * o7 + 1].astype(BF16)
    # [128, block, node] broadcast tiles (replicated across 40 blocks = max
    # stripe width; blend slices only use the replica COUNT, not position)
    db = np.zeros((128, 2 * 40 * 128), BF16)
    db[:, 0:40 * 128] = np.tile(delta, 40)[None, :]
    db[:, 40 * 128:] = np.tile(beta, 40)[None, :]
    return wt, db


def build_nc(rows, k_blk=K, supers=None, dve_cut=DVE_CUT):
    if supers is None:
        supers = SUPERS
    assert rows % (128 * k_blk) == 0
    blocks = rows // 128
    groups = blocks // k_blk
    assert sum(supers) == groups
    bf = mybir.dt.bfloat16
    f32 = mybir.dt.float32
    A = mybir.AluOpType

    nc = bacc.Bacc()
    x_in = nc.dram_tensor("x", [F + 2, rows], bf, kind="ExternalInput")
    wt_in = nc.dram_tensor("wt", [F + 2, 256], bf, kind="ExternalInput")
    db_in = nc.dram_tensor("db", [128, 2 * 40 * 128], bf, kind="ExternalInput")
    out_dram = nc.dram_tensor("out", [128, blocks], bf, kind="ExternalOutput")

    GW = 255 * k_blk  # 2040 cols per group slab

    with tile.TileContext(nc) as tc:
        with (
            tc.tile_pool(name="consts", bufs=1) as consts,
            tc.tile_pool(name="xT", bufs=1) as xtp,
            tc.tile_pool(name="zps", bufs=1, space="PSUM") as zps,
            tc.tile_pool(name="gpool", bufs=3) as gpool,
            tc.tile_pool(name="blend", bufs=2) as blp,
            tc.tile_pool(name="v3pool", bufs=2) as v3p,
            tc.tile_pool(name="opool", bufs=1) as opool,
        ):
            # ---- constants ----
            wt_sb = consts.tile([F + 2, 256], bf)
            nc.gpsimd.dma_start(out=wt_sb[:], in_=wt_in[:])
            dbt = consts.tile([128, 2 * 40 * 128], bf)
            # warmup: preload the sigmoid ACT table before real data arrives
            warm = consts.tile([128, 1], f32)
            nc.vector.memset(warm[:], 0.0)
            wsig = consts.tile([128, 1], bf)
            nc.scalar.activation(out=wsig[:], in_=warm[:],
                                 func=mybir.ActivationFunctionType.Sigmoid)
            dbc = dbt[:, 0:40 * 128].rearrange("p (b n) -> p b n", n=128)
            bbc = dbt[:, 40 * 128:].rearrange("p (b n) -> p b n", n=128)

            # ---- x: feature-major bf16, pooled chunks on the SP queue; the
            # big db broadcast tile loads after the first x chunks ----
            xchunks = []   # (col0, cols, tile)
            off = 0
            n_chunk = 0
            while off < rows:
                cs = 2048 if n_chunk < 2 else 4096
                cs = min(cs, rows - off)
                cxt = xtp.tile([F + 2, cs], bf, tag=f"x{cs}", name=f"xc{n_chunk}",
                               bufs=(2 if cs == 2048 else 4))
                nc.sync.dma_start(out=cxt[:], in_=x_in[:, off:off + cs])
                xchunks.append((off, cs, cxt))
                off += cs
                n_chunk += 1
                if n_chunk == 2:
                    nc.gpsimd.dma_start(out=dbt[:, 0:5120], in_=db_in[:, 0:5120])
                    nc.gpsimd.dma_start(out=dbt[:, 5120:], in_=db_in[:, 5120:])

            def x_slice(b):
                c0 = b * 128
                for off_, cs_, t_ in xchunks:
                    if off_ <= c0 < off_ + cs_:
                        return t_[:, c0 - off_:c0 - off_ + 128]
                raise AssertionError

            out_sb = opool.tile([128, blocks], bf)
            # PE p-state warmup: dummy matmuls keep PE busy during x load
            dmy = consts.tile([128, 512], bf)
            nc.vector.memset(dmy[:], 0.0)
            # persistent PSUM tile; two 2048-col halves alternate per group
            zbig = zps.tile([128, 4096], f32)
            for w in range(5):
                nc.tensor.matmul(zbig[:, 2048:2560], lhsT=dmy[:, 0:128],
                                 rhs=dmy[:], start=True, stop=True)

            def emit_z(g):
                zt = zbig[:, (g % 2) * 2048:(g % 2) * 2048 + GW]
                b0 = g * k_blk
                for j in range(k_blk):
                    xs = x_slice(b0 + j)
                    nc.tensor.matmul(
                        zt[:, 255 * j:255 * (j + 1)],
                        lhsT=xs, rhs=wt_sb[:, 0:255],
                        start=True, stop=True)

            sup_of_group = []
            for si, sg in enumerate(supers):
                sup_of_group += [si] * sg
            g_tiles = {}

            def emit_sig(g):
                si = sup_of_group[g]
                sg = supers[si]
                if si not in g_tiles:
                    g_tiles[si] = gpool.tile(
                        [128, GW * sg], bf, tag=f"g{sg}", name=f"gsup{si}",
                        bufs=(3 if sg == 4 else (1 if sg == 2 else 2)))
                q = g - sup_of_group.index(si)
                zt = zbig[:, (g % 2) * 2048:(g % 2) * 2048 + GW]
                nc.scalar.activation(
                    out=g_tiles[si][:, q * GW:(q + 1) * GW], in_=zt,
                    func=mybir.ActivationFunctionType.Sigmoid)

            def emit_blend(si, g0):
                """blend for super si covering blocks [g0*K, g0*K + sb*K)."""
                sg = supers[si]
                sb = sg * k_blk                      # blocks in this super
                b0 = g0 * k_blk
                gt = g_tiles[si]
                gv = gt[:, 0:sb * 255].rearrange("p (b c) -> p b c", c=255)
                cut = max(1, min(sb - 1, round(dve_cut * sb)))
                stripes = [
                    (nc.vector, 0, cut, "dv"),
                    (nc.gpsimd, cut, sb, "pl"),
                ]
                v3s = v3p.tile([128, sb, 8], bf, tag=f"v3_{sg}", name="v3s", bufs=(2 if sg == 4 else 1))
                for eng, s0, s1, nm in stripes:
                    sw = s1 - s0
                    gs = gv[:, s0:s1, :]
                    # L7: v = g7*delta + beta
                    vt = blp.tile([128, sw, 128], bf, tag=f"v7{nm}_{sg}", name="vt", bufs=(2 if sg == 4 else 1))
                    v = vt[:, 0:sw, :]
                    eng.tensor_tensor(v, gs[:, :, 0:128], dbc[:, 0:sw, :], A.mult)
                    eng.tensor_tensor(v, v, bbc[:, 0:sw, :], A.add)
                    for k in range(6, 2, -1):
                        m = 2 ** k
                        off_ = LEVEL_OFF[k]
                        vl = v[:, :, 0:m]
                        vr = v[:, :, m:2 * m]
                        if k == 3:
                            vn = v3s[:, s0:s1, :]
                        else:
                            vnt = blp.tile([128, sw, m], bf, tag=f"v{k}{nm}_{sg}", name="vnt", bufs=(2 if sg == 4 else 1))
                            vn = vnt[:, 0:sw, :]
                        dtt = blp.tile([128, sw, m], bf, tag=f"d{k}{nm}_{sg}", name="dtt", bufs=(2 if sg == 4 else 1))
                        dt_ = dtt[:, 0:sw, :]
                        eng.tensor_tensor(dt_, vl, vr, A.subtract)
                        eng.tensor_tensor(vn, gs[:, :, off_:off_ + m], dt_, A.mult)
                        eng.tensor_tensor(vn, vn, vr, A.add)
                        v = vn if k > 3 else None
                # levels 2..0 all on Pool
                v = v3s[:, 0:sb, :]
                for k in range(2, -1, -1):
                    m = 2 ** k
                    off_ = LEVEL_OFF[k]
                    vl = v[:, :, 0:m]
                    vr = v[:, :, m:2 * m]
                    dtt = blp.tile([128, sb, m], bf, tag=f"dj{k}_{sg}", name="dtt", bufs=(2 if sg == 4 else 1))
                    dt_ = dtt[:, 0:sb, :]
                    nc.gpsimd.tensor_tensor(dt_, vl, vr, A.subtract)
                    gk = gv[:, :, LEVEL_OFF[k]:LEVEL_OFF[k] + m]
                    if k > 0:
                        vnt = blp.tile([128, sb, m], bf, tag=f"vj{k}_{sg}", name="vnt", bufs=(2 if sg == 4 else 1))
                        vn = vnt[:, 0:sb, :]
                        nc.gpsimd.tensor_tensor(vn, gk, dt_, A.mult)
                        nc.gpsimd.tensor_tensor(vn, vn, vr, A.add)
                        v = vn
                    else:
                        vo = out_sb[:, b0:b0 + sb]
                        g0v = gk.rearrange("p a b -> p (a b)")
                        d0v = dt_.rearrange("p a b -> p (a b)")
                        r0v = vr.rearrange("p a b -> p (a b)")
                        nc.gpsimd.tensor_tensor(vo, g0v, d0v, A.mult)
                        nc.gpsimd.tensor_tensor(vo, vo, r0v, A.add)
                if si < len(supers) - 3:
                    nc.sync.dma_start(out=out_dram[:, b0:b0 + sb],
                                      in_=out_sb[:, b0:b0 + sb])
                elif si == len(supers) - 1:
                    btail = blocks - sum(supers[-3:]) * k_blk
                    nc.sync.dma_start(out=out_dram[:, btail:],
                                      in_=out_sb[:, btail:])

            # software-pipelined emission: z one group ahead of σ
            emit_z(0)
            sup_start = {}
            for g in range(groups):
                si = sup_of_group[g]
                if si not in sup_start:
                    sup_start[si] = g
                if g + 1 < groups:
                    emit_z(g + 1)
                emit_sig(g)
                if g == sup_start[si] + supers[si] - 1:
                    emit_blend(si, sup_start[si])

    return nc


_CACHE = {}


def _get_nc(rows, slots=SLOTS, chunk=CHUNK):
    key = (rows,)
    if key not in _CACHE:
        nc = build_nc(rows)
        if not nc.is_finalized():
            nc.finalize()
        _CACHE[key] = nc
    return _CACHE[key]


def run_device(xT, wt, db, slots=SLOTS, chunk=CHUNK, n_cores=N_CORES, trace=False):
    rows = xT.shape[1] // n_cores
    nc = _get_nc(rows)
    in_maps = [
        {
            "x": np.ascontiguousarray(xT[:, i * rows:(i + 1) * rows]),
            "wt": wt,
            "db": db,
        }
        for i in range(n_cores)
    ]
    res = run_bass_kernel_spmd(nc, in_maps, list(range(n_cores)), trace=trace)
    out = np.empty((n_cores * rows, 1), np.float32)
    for i in range(n_cores):
        dev = res.results[i]["out"].astype(np.float32)  # [128, blocks]
        out[i * rows:(i + 1) * rows, 0] = dev.T.reshape(-1)
    return out, res


def pad_x(x):
    """Host staging: transpose to feature-major, two ones rows, cast bf16."""
    n = x.shape[0]
    xp = np.ones((F + 2, n), np.float32)
    xp[0:F, :] = x.T
    return xp.astype(BF16)


def kernel(**inputs):
    x = np.asarray(inputs["x"], np.float32).reshape(-1, F)
    wt, db = host_prep(
        inputs["feature_importances"],
        inputs["feature_splits"],
        inputs["leaf_node_classes"],
    )
    xT = pad_x(x)
    out, _ = run_device(xT, wt, db)
    return out


# revision 34
# speedup vs baseline: 1.2552x; 1.0006x over previous
"""Soft decision-tree forward (nn_DTree) on 8 trn2 NeuronCores.

Strategy (pure data parallel): shard x row-wise 8 ways, replicate tree params.

Per core (32768 rows = 256 blocks of 128 rows):
  - One 255-col GEMM per block: z = [x|1|1] @ [W | -c_hi | -c_lo]^T into a
    persistent PSUM tile (two 4-bank halves alternate per 8-block group;
    range-granular WAR tracking keeps the next-next group's GEMMs off the
    sigmoid's critical path).
  - ONE sigmoid per group (fe=2040) -> bf16 g in SBUF.  g is laid out at a
    uniform 255-col stride per block across a 32-block super-tile, so every
    blend op batches 32 blocks with a 3D access pattern.
  - Value-tree blend (level-major, left-children-first permutation):
    levels 7..3 striped across DVE (2x bf16) and Pool by block ranges,
    levels 2..0 (tiny) entirely on Pool.  Level 7 blends with broadcast
    delta/beta const tiles; output written bf16.
  - x arrives host-transposed bf16 [34, rows]; output [128, 256] bf16,
    unpacked on host.
"""

import numpy as np
import ml_dtypes

import concourse.bass as bass
import concourse.bacc as bacc
import concourse.tile as tile
from concourse import mybir
from concourse.bass_utils import run_bass_kernel_spmd

BF16 = ml_dtypes.bfloat16

F = 32
D = 8
NODES = 255
LEAVES = 256
N_FULL = 262144
N_CORES = 8
ROWS = N_FULL // N_CORES  # 32768 rows per core
SLOTS = 32                # kept for test.py compat
CHUNK = 4096              # kept for test.py compat

K = 8                                  # blocks per σ-group
SUPERS = [1, 1, 1, 1, 2, 2, 4, 4, 4, 4, 4, 2, 1, 1]  # fine taper
DVE_CUT = 155 / 256                    # fraction of blocks on DVE for L7..3

# level-major offsets of each level's gates inside the 255-col block
LEVEL_OFF = {7: 0, 6: 128, 5: 192, 4: 224, 3: 240, 2: 248, 1: 252, 0: 254}


def _orderings():
    """ord[k] = local node order at level k (left-children-first recursion)."""
    ordv = {0: [0]}
    for k in range(7):
        ordv[k + 1] = [2 * i for i in ordv[k]] + [2 * i + 1 for i in ordv[k]]
    col_nodes = []
    for k in range(7, -1, -1):
        base = 2 ** k - 1
        col_nodes += [base + i for i in ordv[k]]
    return ordv, np.array(col_nodes)


def host_prep(feature_importances, feature_splits, leaf_node_classes, slots=SLOTS):
    """relu/sigmoid/c, node permutation, bf16 weights with split bias rows,
    leaf-blend delta/beta broadcast constants."""
    fi = np.asarray(feature_importances, np.float32).reshape(NODES, F)
    fs = np.asarray(feature_splits, np.float32).reshape(NODES, F)
    cls = np.asarray(leaf_node_classes, np.float32).reshape(LEAVES)

    W = np.maximum(fi, 0.0)
    S = 1.0 / (1.0 + np.exp(-fs))
    c = np.sum(W * S, axis=1)  # (NODES,)

    ordv, col_nodes = _orderings()
    Wp = W[col_nodes]          # (255, 32) permuted level-major
    cp = c[col_nodes]

    c_hi = cp.astype(BF16).astype(np.float32)
    c_lo = (cp - c_hi).astype(np.float32)

    wt = np.zeros((F + 2, 256), BF16)
    wt[0:F, 0:NODES] = Wp.T.astype(BF16)
    wt[F, 0:NODES] = (-c_hi).astype(BF16)
    wt[F + 1, 0:NODES] = (-c_lo).astype(BF16)

    o7 = np.array(ordv[7])
    delta = (cls[2 * o7] - cls[2 * o7 + 1]).astype(BF16)
    beta = cls[2 * o7 + 1].astype(BF16)
    # [128, block, node] broadcast tiles (replicated across 20 blocks = max
    # stripe width; blend slices only use the replica COUNT, not position)
    db = np.zeros((128, 2 * 20 * 128), BF16)
    db[:, 0:20 * 128] = np.tile(delta, 20)[None, :]
    db[:, 20 * 128:] = np.tile(beta, 20)[None, :]
    return wt, db


def build_nc(rows, k_blk=K, supers=None, dve_cut=DVE_CUT):
    if supers is None:
        supers = SUPERS
    assert rows % (128 * k_blk) == 0
    blocks = rows // 128
    groups = blocks // k_blk
    assert sum(supers) == groups
    bf = mybir.dt.bfloat16
    f32 = mybir.dt.float32
    A = mybir.AluOpType

    nc = bacc.Bacc()
    x_in = nc.dram_tensor("x", [F + 2, rows], bf, kind="ExternalInput")
    wt_in = nc.dram_tensor("wt", [F + 2, 256], bf, kind="ExternalInput")
    db_in = nc.dram_tensor("db", [128, 2 * 20 * 128], bf, kind="ExternalInput")
    out_dram = nc.dram_tensor("out", [128, blocks], bf, kind="ExternalOutput")

    GW = 255 * k_blk  # 2040 cols per group slab

    with tile.TileContext(nc) as tc:
        with (
            tc.tile_pool(name="consts", bufs=1) as consts,
            tc.tile_pool(name="xT", bufs=1) as xtp,
            tc.tile_pool(name="zps", bufs=1, space="PSUM") as zps,
            tc.tile_pool(name="gpool", bufs=3) as gpool,
            tc.tile_pool(name="blend", bufs=2) as blp,
            tc.tile_pool(name="v3pool", bufs=2) as v3p,
            tc.tile_pool(name="opool", bufs=1) as opool,
        ):
            # ---- constants ----
            wt_sb = consts.tile([F + 2, 256], bf)
            nc.gpsimd.dma_start(out=wt_sb[:], in_=wt_in[:])
            dbt = consts.tile([128, 2 * 20 * 128], bf)
            # warmup: preload the sigmoid ACT table before real data arrives
            warm = consts.tile([128, 1], f32)
            nc.vector.memset(warm[:], 0.0)
            wsig = consts.tile([128, 1], bf)
            nc.scalar.activation(out=wsig[:], in_=warm[:],
                                 func=mybir.ActivationFunctionType.Sigmoid)
            dbc = dbt[:, 0:20 * 128].rearrange("p (b n) -> p b n", n=128)
            bbc = dbt[:, 20 * 128:].rearrange("p (b n) -> p b n", n=128)

            # ---- x: feature-major bf16, pooled chunks on the SP queue; the
            # big db broadcast tile loads after the first x chunks ----
            xchunks = []   # (col0, cols, tile)
            off = 0
            n_chunk = 0
            while off < rows:
                cs = 2048 if n_chunk < 2 else 4096
                cs = min(cs, rows - off)
                cxt = xtp.tile([F + 2, cs], bf, tag=f"x{cs}", name=f"xc{n_chunk}",
                               bufs=2)
                nc.sync.dma_start(out=cxt[:], in_=x_in[:, off:off + cs])
                xchunks.append((off, cs, cxt))
                off += cs
                n_chunk += 1
                if n_chunk == 2:
                    nc.gpsimd.dma_start(out=dbt[:, 0:2560], in_=db_in[:, 0:2560])
                    nc.gpsimd.dma_start(out=dbt[:, 2560:], in_=db_in[:, 2560:])

            def x_slice(b):
                c0 = b * 128
                for off_, cs_, t_ in xchunks:
                    if off_ <= c0 < off_ + cs_:
                        return t_[:, c0 - off_:c0 - off_ + 128]
                raise AssertionError

            out_sb = opool.tile([128, blocks], bf)
            # PE p-state warmup: dummy matmuls keep PE busy during x load
            dmy = consts.tile([128, 512], bf)
            nc.vector.memset(dmy[:], 0.0)
            # persistent PSUM tile; two 2048-col halves alternate per group
            zbig = zps.tile([128, 4096], f32)
            for w in range(5):
                nc.tensor.matmul(zbig[:, 2048:2560], lhsT=dmy[:, 0:128],
                                 rhs=dmy[:], start=True, stop=True)

            def emit_z(g):
                zt = zbig[:, (g % 2) * 2048:(g % 2) * 2048 + GW]
                b0 = g * k_blk
                for j in range(k_blk):
                    xs = x_slice(b0 + j)
                    nc.tensor.matmul(
                        zt[:, 255 * j:255 * (j + 1)],
                        lhsT=xs, rhs=wt_sb[:, 0:255],
                        start=True, stop=True)

            sup_of_group = []
            for si, sg in enumerate(supers):
                sup_of_group += [si] * sg
            g_tiles = {}

            def emit_sig(g):
                si = sup_of_group[g]
                sg = supers[si]
                if si not in g_tiles:
                    g_tiles[si] = gpool.tile(
                        [128, GW * sg], bf, tag=f"g{sg}", name=f"gsup{si}",
                        bufs=(3 if sg == 4 else (2 if sg == 2 else 4)))
                q = g - sup_of_group.index(si)
                zt = zbig[:, (g % 2) * 2048:(g % 2) * 2048 + GW]
                nc.scalar.activation(
                    out=g_tiles[si][:, q * GW:(q + 1) * GW], in_=zt,
                    func=mybir.ActivationFunctionType.Sigmoid)

            def emit_blend(si, g0):
                """blend for super si covering blocks [g0*K, g0*K + sb*K)."""
                sg = supers[si]
                sb = sg * k_blk                      # blocks in this super
                b0 = g0 * k_blk
                gt = g_tiles[si]
                gv = gt[:, 0:sb * 255].rearrange("p (b c) -> p b c", c=255)
                cut = max(1, min(sb - 1, round(dve_cut * sb)))
                stripes = [
                    (nc.vector, 0, cut, "dv"),
                    (nc.gpsimd, cut, sb, "pl"),
                ]
                v3s = v3p.tile([128, sb, 8], bf, tag=f"v3_{sg}", name="v3s", bufs=(2 if sg != 2 else 1))
                for eng, s0, s1, nm in stripes:
                    sw = s1 - s0
                    gs = gv[:, s0:s1, :]
                    # L7: v = g7*delta + beta
                    vt = blp.tile([128, sw, 128], bf, tag=f"v7{nm}_{sg}", name="vt", bufs=(2 if sg != 2 else 1))
                    v = vt[:, 0:sw, :]
                    eng.tensor_tensor(v, gs[:, :, 0:128], dbc[:, 0:sw, :], A.mult)
                    eng.tensor_tensor(v, v, bbc[:, 0:sw, :], A.add)
                    for k in range(6, 2, -1):
                        m = 2 ** k
                        off_ = LEVEL_OFF[k]
                        vl = v[:, :, 0:m]
                        vr = v[:, :, m:2 * m]
                        if k == 3:
                            vn = v3s[:, s0:s1, :]
                        else:
                            vnt = blp.tile([128, sw, m], bf, tag=f"v{k}{nm}_{sg}", name="vnt", bufs=(2 if sg != 2 else 1))
                            vn = vnt[:, 0:sw, :]
                        dtt = blp.tile([128, sw, m], bf, tag=f"d{k}{nm}_{sg}", name="dtt", bufs=(2 if sg != 2 else 1))
                        dt_ = dtt[:, 0:sw, :]
                        eng.tensor_tensor(dt_, vl, vr, A.subtract)
                        eng.tensor_tensor(vn, gs[:, :, off_:off_ + m], dt_, A.mult)
                        eng.tensor_tensor(vn, vn, vr, A.add)
                        v = vn if k > 3 else None
                # levels 2..0 all on Pool
                v = v3s[:, 0:sb, :]
                for k in range(2, -1, -1):
                    m = 2 ** k
                    off_ = LEVEL_OFF[k]
                    vl = v[:, :, 0:m]
                    vr = v[:, :, m:2 * m]
                    dtt = blp.tile([128, sb, m], bf, tag=f"dj{k}_{sg}", name="dtt", bufs=(2 if sg != 2 else 1))
                    dt_ = dtt[:, 0:sb, :]
                    nc.gpsimd.tensor_tensor(dt_, vl, vr, A.subtract)
                    gk = gv[:, :, LEVEL_OFF[k]:LEVEL_OFF[k] + m]
                    if k > 0:
                        vnt = blp.tile([128, sb, m], bf, tag=f"vj{k}_{sg}", name="vnt", bufs=(2 if sg != 2 else 1))
                        vn = vnt[:, 0:sb, :]
                        nc.gpsimd.tensor_tensor(vn, gk, dt_, A.mult)
                        nc.gpsimd.tensor_tensor(vn, vn, vr, A.add)
                        v = vn
                    else:
                        vo = out_sb[:, b0:b0 + sb]
                        g0v = gk.rearrange("p a b -> p (a b)")
                        d0v = dt_.rearrange("p a b -> p (a b)")
                        r0v = vr.rearrange("p a b -> p (a b)")
                        nc.gpsimd.tensor_tensor(vo, g0v, d0v, A.mult)
                        nc.gpsimd.tensor_tensor(vo, vo, r0v, A.add)
                if si < len(supers) - 3:
                    nc.sync.dma_start(out=out_dram[:, b0:b0 + sb],
                                      in_=out_sb[:, b0:b0 + sb])
                elif si == len(supers) - 1:
                    btail = blocks - sum(supers[-3:]) * k_blk
                    nc.sync.dma_start(out=out_dram[:, btail:],
                                      in_=out_sb[:, btail:])

            # software-pipelined emission: z one group ahead of σ
            emit_z(0)
            sup_start = {}
            for g in range(groups):
                si = sup_of_group[g]
                if si not in sup_start:
                    sup_start[si] = g
                if g + 1 < groups:
                    emit_z(g + 1)
                emit_sig(g)
                if g == sup_start[si] + supers[si] - 1:
                    emit_blend(si, sup_start[si])

    return nc


_CACHE = {}


def _get_nc(rows, slots=SLOTS, chunk=CHUNK):
    key = (rows,)
    if key not in _CACHE:
        nc = build_nc(rows)
        if not nc.is_finalized():
            nc.finalize()
        _CACHE[key] = nc
    return _CACHE[key]


def run_device(xT, wt, db, slots=SLOTS, chunk=CHUNK, n_cores=N_CORES, trace=False):
    rows = xT.shape[1] // n_cores
    nc = _get_nc(rows)
    in_maps = [
        {
            "x": np.ascontiguousarray(xT[:, i * rows:(i + 1) * rows]),
            "wt": wt,
            "db": db,
        }
        for i in range(n_cores)
    ]
    res = run_bass_kernel_spmd(nc, in_maps, list(range(n_cores)), trace=trace)
    out = np.empty((n_cores * rows, 1), np.float32)
    for i in range(n_cores):
        dev = res.results[i]["out"].astype(np.float32)  # [128, blocks]
        out[i * rows:(i + 1) * rows, 0] = dev.T.reshape(-1)
    return out, res


def pad_x(x):
    """Host staging: transpose to feature-major, two ones rows, cast bf16."""
    n = x.shape[0]
    xp = np.ones((F + 2, n), np.float32)
    xp[0:F, :] = x.T
    return xp.astype(BF16)


def kernel(**inputs):
    x = np.asarray(inputs["x"], np.float32).reshape(-1, F)
    wt, db = host_prep(
        inputs["feature_importances"],
        inputs["feature_splits"],
        inputs["leaf_node_classes"],
    )
    xT = pad_x(x)
    out, _ = run_device(xT, wt, db)
    return out
